# revision 38
# baseline (speedup 1.0000x reference)
"""Trainium2 Bass kernel for BaseModel.forgetting_norm.

Math (per batch b):
    m[t]  = mean over 514 channel*freq rows of x[b, :, t]
    mu[t] = alp[t] * mu[t-1] + (1 - alp[t]) * m[t]          (EMA over time)
    out[b, cf, t] = x[b, cf, t] / (mu[t] + 1e-10)

Mapping (pure data parallel, batch 32 -> 4 per core on 8 cores). The
problem is HBM-bound (16.4 MB in + 16.4 MB out per core ~ 92 us at
358 GB/s), so everything else is arranged to hide under the DMA:
  - x loaded with an fp32 -> bf16 cast in the DMA (SWDGE), one
    [128, 4x2000] tile per batch (rows 4p..4p+3 on partition p, 32 KB
    contiguous per partition; rel tolerance is 2e-2, bf16 costs ~0.2%).
    Ragged rows (514 = 4*128+2) get their own [2, 2000] tile.
  - channel sums via TensorE in bf16 (1 cyc/col vs 4 for fp32):
    ones[128,1] x chunk -> PSUM [1,500] per t-chunk, accumulated over
    the 4 row-groups + ragged rows; (1-alp)/514 folded in by the DVE
    copy-out. PSUM mean tiles are per-t-half and double-buffered so the
    next batch's matmuls overlap this batch's DVE chain.
  - EMA via one VectorE tensor_tensor_scan per batch (fp32 state),
    reciprocal via reciprocal_approx_fast, both in-place.
  - broadcast of the reciprocal row across 128 partitions via rank-1
    bf16 matmul; ScalarE copies it PSUM -> SBUF as bf16 so the divides
    run as all-SBUF bf16 tensor_tensor (2x DVE mode).
  - stores cast bf16 -> fp32 during the DMA (SWDGE).
  - dummy matmuls during the load phase pre-warm the PE past the HAM
    clock gate so batch 0's mean matmuls run at 2.4 GHz, shortening the
    pipeline head before the first store.
"""

import sys

sys.path.insert(0, "/opt/trn_rl_repo")

import numpy as np

import concourse.bass as bass
import concourse.bacc as bacc
import concourse.tile as tile
from concourse import mybir
from concourse.bass_utils import run_bass_kernel_spmd

B, C, F, T = 32, 2, 257, 2000
CF = C * F  # 514
NCORES = 8
BL = B // NCORES  # 4 batches per core
NFULL = 4  # 512 = 128 * 4 rows in the main tile
RAG = CF - 128 * NFULL  # 2 ragged cf rows
HW = T // 2  # 1000, t-half width


def _build_kernel(nc: bass.Bass, tc_: tile.TileContext, ctx):
    f32 = mybir.dt.float32
    bf16 = mybir.dt.bfloat16
    x = nc.dram_tensor("x", [BL, CF, T], f32, kind="ExternalInput").ap()
    alp = nc.dram_tensor("alp", [1, T], f32, kind="ExternalInput").ap()
    c14 = nc.dram_tensor("c14", [1, T], f32, kind="ExternalInput").ap()
    out = nc.dram_tensor("out", [BL, CF, T], f32, kind="ExternalOutput").ap()

    consts = ctx.enter_context(tc_.tile_pool(name="consts", bufs=1))
    xpool = ctx.enter_context(tc_.tile_pool(name="xpool", bufs=4))
    ragp = ctx.enter_context(tc_.tile_pool(name="ragp", bufs=4))
    rows = ctx.enter_context(tc_.tile_pool(name="rows", bufs=2))
    rbc16p = ctx.enter_context(tc_.tile_pool(name="rbc16", bufs=2))
    mpsum = ctx.enter_context(tc_.tile_pool(name="mpsum", bufs=2, space="PSUM"))
    rbcp = ctx.enter_context(tc_.tile_pool(name="rbcp", bufs=2, space="PSUM"))

    ones_bf = consts.tile([128, 1], bf16)
    nc.vector.memset(ones_bf, 1.0)
    ones_row = consts.tile([1, 128], bf16)
    nc.vector.memset(ones_row, 1.0)
    alp_sb = consts.tile([1, T], f32)
    nc.sync.dma_start(out=alp_sb, in_=alp)
    c14_sb = consts.tile([1, T], f32)
    nc.sync.dma_start(out=c14_sb, in_=c14)

    # ---- loads (fp32 -> bf16 cast in the DMA, SWDGE) ----
    # batch 0's load is split in two so its mean matmuls start earlier
    # (shortens the pipeline head before the first store).
    xt, rg = [], []
    for b in range(BL):
        t_ = xpool.tile([128, NFULL, T], bf16, tag="xt")
        src = x[b, 0 : 128 * NFULL, :].rearrange("(p j) t -> p j t", j=NFULL)
        if b == 0:
            for j in range(NFULL):
                nc.gpsimd.dma_start(
                    out=t_[:, j : j + 1, :], in_=src[:, j : j + 1, :]
                )
        else:
            nc.gpsimd.dma_start(out=t_, in_=src)
        xt.append(t_)
        r_ = ragp.tile([RAG, T], bf16, tag="rag")
        nc.gpsimd.dma_start(out=r_, in_=x[b, 128 * NFULL :, :])
        rg.append(r_)

    # ---- PE warm-up: the HAM clock gate halves the PE clock until it
    # sees ~3.5 us of sustained activity. The PE is idle during the load
    # phase anyway, so run dummy matmuls on const tiles (results never
    # read) so batch 0's mean matmuls start at the warm clock.
    dummy_row = consts.tile([1, 512], bf16)
    nc.vector.memset(dummy_row, 1.0)
    warm = rbcp.tile([128, 1024], f32, tag="rbc")
    for w in range(12):
        nc.tensor.matmul(
            warm[:, (w % 2) * 512 : (w % 2) * 512 + 500],
            ones_row[0:1, :],
            dummy_row[:, 0:500],
            start=(w < 2),
            stop=(w >= 10),
        )

    rrrs = {}

    def means_and_rowchain(b):
        # ---- channel sums (TensorE, bf16 -> fp32 PSUM), per t-half ----
        mu = rows.tile([1, T], f32, tag="mu")
        for h in range(2):
            mh = mpsum.tile([1, 2, 512], f32, tag="mh")
            for j in range(NFULL):
                for c in range(2):
                    t0 = h * HW + c * 500
                    nc.tensor.matmul(
                        mh[:, c, 0:500],
                        ones_bf[:, 0:1],
                        xt[b][:, j, t0 : t0 + 500],
                        start=(j == 0),
                        stop=False,
                    )
            for c in range(2):
                t0 = h * HW + c * 500
                nc.tensor.matmul(
                    mh[:, c, 0:500],
                    ones_bf[0:RAG, 0:1],
                    rg[b][:, t0 : t0 + 500],
                    start=False,
                    stop=True,
                )
            # EMA input b[t] = (1-alp[t])/514 * sum[t] (DVE reads PSUM)
            nc.vector.tensor_mul(
                mu[:, h * HW : (h + 1) * HW].rearrange("p (a s) -> p a s", a=2),
                mh[:, :, 0:500],
                c14_sb[:, h * HW : (h + 1) * HW].rearrange("p (a s) -> p a s", a=2),
            )

        # ---- EMA scan (in place): state = alp*state + b ----
        nc.vector.tensor_tensor_scan(
            mu, alp_sb, mu, 0.0, mybir.AluOpType.mult, mybir.AluOpType.add
        )
        # mu >= ~0.25 * min-mean, so skipping the reference's +1e-10 is a
        # ~1e-10 relative difference; approx reciprocal is ~51 ULP.
        nc.vector.reciprocal_approx_fast(out=mu, in_=mu)
        # bf16 here costs nothing extra: rbc16 (the divide operand) is
        # already bf16, and bf16(bf16(r)) == bf16(r).
        rrr = rows.tile([1, T], bf16, tag="rrr")
        nc.scalar.copy(out=rrr, in_=mu)
        rrrs[b] = rrr

    def finish(b):
        rrr = rrrs[b]
        # ---- broadcast across 128 partitions; PSUM -> SBUF as bf16 ----
        rbc16 = rbc16p.tile([128, T], bf16, tag="rbc16")
        for h in range(2):
            rbc = rbcp.tile([128, 1024], f32, tag="rbc")
            for s, w in ((0, 512), (512, 488)):
                nc.tensor.matmul(
                    rbc[:, s : s + w],
                    ones_row[0:1, :],
                    rrr[:, h * HW + s : h * HW + s + w],
                    start=True,
                    stop=True,
                )
            nc.scalar.copy(out=rbc16[:, h * HW : (h + 1) * HW], in_=rbc[:, 0:HW])

        # ---- divides (all-SBUF bf16 tensor_tensor, 2x mode); the
        # reciprocal row is stride-0 broadcast over the 4 row-groups so
        # the whole tile divides in one instruction ----
        nc.vector.tensor_mul(
            xt[b],
            xt[b],
            rbc16.rearrange("p (o t) -> p o t", o=1).broadcast_to([128, NFULL, T]),
        )
        nc.vector.tensor_mul(rg[b], rg[b], rbc16[0:RAG, :])

        # ---- stores (bf16 -> fp32 cast in the DMA) ----
        nc.gpsimd.dma_start(
            out=out[b, 0 : 128 * NFULL, :].rearrange("(p j) t -> p j t", j=NFULL),
            in_=xt[b],
        )
        nc.gpsimd.dma_start(out=out[b, 128 * NFULL :, :], in_=rg[b])

    # Software-pipelined issue order: batch b+1's mean matmuls are issued
    # before batch b's broadcast/divide/store so the PE queue never stalls
    # the next batch behind a broadcast that waits on the DVE row chain.
    for b in range(BL):
        means_and_rowchain(b)
        if b >= 1:
            finish(b - 1)
    finish(BL - 1)


_NC_CACHE = None


def build_bass() -> bass.Bass:
    global _NC_CACHE
    if _NC_CACHE is not None:
        return _NC_CACHE
    import contextlib

    nc = bacc.Bacc("TRN2", debug=False, enable_asserts=True, num_devices=NCORES)
    with tile.TileContext(nc) as tc_:
        with contextlib.ExitStack() as ctx:
            _build_kernel(nc, tc_, ctx)
    nc.compile()
    _NC_CACHE = nc
    return nc


def host_coeffs(sample_length: int):
    """alp[t] exactly as the reference computes it (fp32 ops), plus the
    folded EMA input coefficient (1-alp)/CF."""
    L = int(sample_length)
    alpha = np.float32((L - 1) / (L + 1))
    idx = np.arange(T, dtype=np.float32)
    one = np.float32(1.0)
    alp = np.minimum((idx - one) / (idx + one), alpha).astype(np.float32)
    c14 = ((one - alp) / np.float32(CF)).astype(np.float32)
    return alp.reshape(1, T), c14.reshape(1, T)


def make_in_maps(x: np.ndarray, sample_length) -> list:
    x = np.ascontiguousarray(np.asarray(x, dtype=np.float32)).reshape(B, CF, T)
    alp, c14 = host_coeffs(int(sample_length))
    return [
        {"x": x[i * BL : (i + 1) * BL], "alp": alp, "c14": c14}
        for i in range(NCORES)
    ]


def kernel(input: np.ndarray, sample_length) -> np.ndarray:
    in_maps = make_in_maps(input, sample_length)
    nc = build_bass()
    res = run_bass_kernel_spmd(nc, in_maps, core_ids=list(range(NCORES)))
    full = np.concatenate([r["out"] for r in res.results], axis=0)
    return full.reshape(B, C, F, T)


if __name__ == "__main__":
    rng = np.random.default_rng(0)
    x = rng.random((B, C, F, T), dtype=np.float32)
    y = kernel(x, 192)
    print(y.shape, y.dtype)
